# revision 1
# baseline (speedup 1.0000x reference)
"""Trainium2 Bass kernel for nn_Decay (gated decay-memory block).

  gate  = sigmoid(x @ Wg + bg)
  store = (x @ Wv) * gate * scale          scale = sqrt(1 - decay)
  mem   = decay-scan(store)                y_t = store_t + decay * y_{t-1}
  que   = sigmoid(x @ Wq + bq)
  out   = (mem * que * scale) @ Wo

Sharding (8 cores): core c handles batch b = c//2, token half h = c%2
(2048 output tokens each).  The decay scan needs history: each core
computes a 512-token halo before its token range (zero-padded for h=0,
so all cores run the identical program).  decay^512 ~ 4e-12 => exact to
fp32 precision.  No collectives.

Layout: everything on-chip lives as [feature (partitions), token (free)].
 - projections:  out[m_tile, t_blk] = sum_ec Wx[ec, m_tile].T @ xT[ec, t_blk]
   (weights in natural [E, M] layout; x transposed host-side)
 - decay scan: DVE tensor_tensor_scan along the free (token) axis
 - output proj consumes load0 [m, t] directly; result is outT [E, T],
   un-transposed host-side.
Matmuls run in float32r (TF32-like, full PE rate at N>=256).

Phases (weights resident in SBUF, activations streamed):
  A0..A3: m-quarter q of {Wv*scale, Wg, Wq} resident; computes
          pv, gate, store, mem(scan), que, load0 -> spill load0 (f32r)
  C0..C3: e-quarter of Wo*scale resident; outT[e, t] = sum_m Wo.T @ load0
Weight tiles use separate tags; the first-needed one (wv / wo) has
bufs=2 so the next phase's load overlaps the current phase's compute.
"""

import sys, types

sys.path.insert(0, "/opt/trn_rl_repo")

import numpy as np

import concourse.bass as bass
import concourse.tile as tile
from concourse import bacc, mybir
from concourse.bass_utils import run_bass_kernel_spmd

# Problem constants (hardcoded per harness contract)
B, S, E, M = 4, 4096, 2048, 2048
DECAY = 0.95
SCALE = float(np.sqrt(1.0 - DECAY))

N_CORES = 8
HALO = 256            # halo tokens ahead of each core's range (decay^256 ~ 2e-6)
OUT_T = S // 2        # output tokens per core
T = OUT_T + HALO      # computed tokens per core
TB = 256              # token block (matmul free dim)
NTB = T // TB         # 10
OTB = OUT_T // TB     # 8 output-token blocks
OTB0 = HALO // TB     # first t-block holding output tokens
P = 128
EC = E // P           # 16 contraction chunks
MT = M // P           # 16 m tiles
MQ = 4                # m-quarters
MT_Q = MT // MQ       # 4 m-tiles per quarter
MQW = MT_Q * P        # 512
F32 = mybir.dt.float32
F32R = mybir.dt.float32r


def build_module(has_bias):
    nc = bacc.Bacc()

    xT_d = nc.dram_tensor("xT", [E, T], F32R, kind="ExternalInput")
    wv_d = nc.dram_tensor("Wvs", [E, M], F32R, kind="ExternalInput")
    wg_d = nc.dram_tensor("Wg", [E, M], F32R, kind="ExternalInput")
    wq_d = nc.dram_tensor("Wq", [E, M], F32R, kind="ExternalInput")
    wo_d = nc.dram_tensor("Wos", [M, E], F32R, kind="ExternalInput")
    if has_bias:
        bg_d = nc.dram_tensor("bg", [M], F32, kind="ExternalInput")
        bq_d = nc.dram_tensor("bq", [M], F32, kind="ExternalInput")
    outT_d = nc.dram_tensor("outT", [E, OUT_T], F32, kind="ExternalOutput")
    l0_buf = nc.dram_tensor("l0_buf", [M, OUT_T], F32R)  # internal spill

    with tile.TileContext(nc) as tc:
        with (
            tc.tile_pool(name="wva", bufs=2) as wvp,   # Wv quarters + Wo quarters
            tc.tile_pool(name="wgap", bufs=2) as wgap,  # Wg half-quarters (prefetch)
            tc.tile_pool(name="wgbp", bufs=1) as wgbp,
            tc.tile_pool(name="wqp", bufs=1) as wqp,
            tc.tile_pool(name="big", bufs=2) as bigp,
            tc.tile_pool(name="ws", bufs=4) as wsp,
            tc.tile_pool(name="mems", bufs=2) as memp,
            tc.tile_pool(name="consts", bufs=1) as cp,
            tc.tile_pool(name="ps", bufs=2, space="PSUM") as ps,
        ):
            # consts: decay broadcast [:, :TB]; bg at [:, TB:TB+MT]; bq after
            consts = cp.tile([P, TB + 2 * MT], F32, tag="consts")
            nc.vector.memset(consts[:, 0:TB], DECAY)
            if has_bias:
                nc.sync.dma_start(
                    out=consts[:, TB : TB + MT],
                    in_=bg_d.rearrange("(c p) -> p c", p=P),
                )
                nc.sync.dma_start(
                    out=consts[:, TB + MT : TB + 2 * MT],
                    in_=bq_d.rearrange("(c p) -> p c", p=P),
                )
            decay_t = consts[:, 0:TB]

            xT_r = xT_d.rearrange("(c p) t -> p c t", p=P)
            l0_r = l0_buf.rearrange("(c p) t -> p c t", p=P)
            outT_r = outT_d.rearrange("(c p) t -> p c t", p=P)

            EQ = E // MQ  # 512
            ETQ = EQ // P  # 4 e-tiles per quarter
            wv_tiles = {}

            def load_wv(q):
                # Wv quarter; prefetched one phase ahead (tag has bufs=2)
                msl = slice(q * MQW, (q + 1) * MQW)
                t = wvp.tile([P, EC, MQW], F32R, tag="wv", name=f"wv{q}")
                nc.scalar.dma_start(
                    out=t, in_=wv_d[:, msl].rearrange("(c p) m -> p c m", p=P)
                )
                wv_tiles[("v", q)] = t

            def load_wo(eq):
                # Wo e-quarter; shares the wv tag / prefetch slot
                esl = slice(eq * EQ, (eq + 1) * EQ)
                t = wvp.tile([P, MT, EQ], F32R, tag="wv", name=f"wo{eq}")
                nc.scalar.dma_start(
                    out=t, in_=wo_d[:, esl].rearrange("(c p) e -> p c e", p=P)
                )
                wv_tiles[("o", eq)] = t

            def load_wga(q):
                # first half (m-tiles 0-1) of the Wg quarter; prefetched
                msl = slice(q * MQW, q * MQW + 2 * P)
                t = wgap.tile([P, EC, 2 * P], F32R, tag="wga", name=f"wga{q}")
                nc.scalar.dma_start(
                    out=t, in_=wg_d[:, msl].rearrange("(c p) m -> p c m", p=P)
                )
                wv_tiles[("gA", q)] = t

            # ---- Phases A0..A3: quarter q of m ----
            for q in range(MQ):
                msl = slice(q * MQW, (q + 1) * MQW)
                if q == 0:
                    # startup: first x block, then weights in order of need
                    xt0 = bigp.tile([P, EC, TB], F32R, tag="t16", name="xt0_0")
                    nc.sync.dma_start(out=xt0, in_=xT_r[:, :, 0:TB])
                    load_wv(0)
                    load_wga(0)
                wv = wv_tiles[("v", q)]
                wga = wv_tiles[("gA", q)]
                mem_prev = None
                for tb in range(NTB):
                    tsl = slice(tb * TB, (tb + 1) * TB)
                    if q == 0 and tb == 0:
                        xt = xt0
                    else:
                        xt = bigp.tile([P, EC, TB], F32R, tag="t16", name=f"xt{q}_{tb}")
                        nc.sync.dma_start(out=xt, in_=xT_r[:, :, tsl])
                    if tb == 0:
                        # rest of this phase's weights (after xt in queue order)
                        mslB = slice(q * MQW + 2 * P, (q + 1) * MQW)
                        wgb = wgbp.tile([P, EC, 2 * P], F32R, tag="wgb", name=f"wgb{q}")
                        nc.sync.dma_start(
                            out=wgb,
                            in_=wg_d[:, mslB].rearrange("(c p) m -> p c m", p=P),
                        )
                    if tb == 1:
                        # wq isn't needed until the first need_q block; delaying
                        # its load keeps boundary bandwidth for wgb/xt
                        wq = wqp.tile([P, EC, MQW], F32R, tag="wq", name=f"wq{q}")
                        nc.gpsimd.dma_start(
                            out=wq, in_=wq_d[:, msl].rearrange("(c p) m -> p c m", p=P)
                        )
                    if tb == 4:
                        # software prefetch of the next phase's first weights
                        if q + 1 < MQ:
                            load_wv(q + 1)
                            load_wga(q + 1)
                        else:
                            load_wo(0)
                    need_q = tb >= OTB0
                    mem_t = memp.tile([P, MT_Q, TB], F32, tag="mem", name=f"mem{q}_{tb}")
                    pvs = []
                    for mt in range(MT_Q):
                        pv = ps.tile(
                            [P, TB], F32, tag="pv", bufs=5, name=f"pv{q}_{tb}_{mt}"
                        )
                        for ec in range(EC):
                            nc.tensor.matmul(
                                pv, lhsT=wv[:, ec, wsl := slice(mt * P, (mt + 1) * P)],
                                rhs=xt[:, ec, :],
                                start=(ec == 0), stop=(ec == EC - 1),
                            )
                        pvs.append(pv)
                    wss = []
                    for mt in range(MT_Q):
                        mtg = q * MT_Q + mt  # global m tile
                        wsl = slice((mt % 2) * P, (mt % 2 + 1) * P)
                        wgt = wga if mt < 2 else wgb
                        ws = wsp.tile([P, 3, TB], F32R, tag="ws", name=f"ws{q}_{tb}_{mt}")
                        wss.append(ws)
                        gate, store = ws[:, 0, :], ws[:, 1, :]
                        pg = ps.tile([P, TB], F32, tag="pg", name=f"pg{q}_{tb}_{mt}")
                        for ec in range(EC):
                            nc.tensor.matmul(
                                pg, lhsT=wgt[:, ec, wsl], rhs=xt[:, ec, :],
                                start=(ec == 0), stop=(ec == EC - 1),
                            )
                        nc.scalar.activation(
                            gate, pg, mybir.ActivationFunctionType.Sigmoid,
                            bias=consts[:, TB + mtg : TB + mtg + 1] if has_bias else 0.0,
                        )
                        nc.vector.tensor_mul(store, pvs[mt], gate)
                        nc.vector.tensor_tensor_scan(
                            mem_t[:, mt, :], decay_t, store,
                            initial=0.0 if tb == 0 else mem_prev[:, mt, TB - 1 : TB],
                            op0=mybir.AluOpType.mult, op1=mybir.AluOpType.add,
                        )
                    if need_q:
                        osl = slice((tb - OTB0) * TB, (tb - OTB0 + 1) * TB)
                        for mt in range(MT_Q):
                            mtg = q * MT_Q + mt
                            wsl = slice(mt * P, (mt + 1) * P)
                            ws = wss[mt]
                            que, l0 = ws[:, 2, :], ws[:, 0, :]  # l0 reuses gate slot
                            pq = ps.tile(
                                [P, TB], F32, tag="pv", bufs=5, name=f"pq{q}_{tb}_{mt}"
                            )
                            for ec in range(EC):
                                nc.tensor.matmul(
                                    pq, lhsT=wq[:, ec, wsl], rhs=xt[:, ec, :],
                                    start=(ec == 0), stop=(ec == EC - 1),
                                )
                            nc.scalar.activation(
                                que, pq, mybir.ActivationFunctionType.Sigmoid,
                                bias=consts[:, TB + MT + mtg : TB + MT + mtg + 1]
                                if has_bias else 0.0,
                            )
                            nc.vector.tensor_mul(l0, mem_t[:, mt, :], que)
                            nc.gpsimd.dma_start(
                                out=l0_r[:, mtg : mtg + 1, osl],
                                in_=l0.unsqueeze(1),
                            )
                    mem_prev = mem_t

            # ---- Phases C: output projection, e-quarter PAIRS resident ----
            # token-block outer so each l0 block is read once per pair
            # wo1 borrows the (now free) wq slot so it loads during cp0's
            # first block instead of waiting for a wv-tag slot
            wo1 = wqp.tile([P, MT, EQ], F32R, tag="wq", name="wo1")
            nc.scalar.dma_start(
                out=wo1, in_=wo_d[:, EQ : 2 * EQ].rearrange("(c p) e -> p c e", p=P)
            )
            wv_tiles[("o", 1)] = wo1
            for cp in range(2):
                eqs = (2 * cp, 2 * cp + 1)
                wos = [wv_tiles[("o", eq)] for eq in eqs]
                for tb in range(OTB):
                    if cp == 0 and tb == 1:
                        load_wo(2)  # wv(A3)'s slot is free -> loads during cp0
                    if cp == 0 and tb == 4:
                        load_wo(3)  # second wv-tag slot frees at wo0... queued
                    tsl = slice(tb * TB, (tb + 1) * TB)
                    lt = bigp.tile([P, MT, TB], F32R, tag="t16", name=f"lt{cp}_{tb}")
                    nc.sync.dma_start(out=lt, in_=l0_r[:, :, tsl])
                    for j, eq in enumerate(eqs):
                        ot = memp.tile(
                            [P, ETQ, TB], F32, tag=("mem", "l0")[j],
                            name=f"ot{eq}_{tb}",
                        )
                        for et in range(ETQ):
                            po = ps.tile([P, TB], F32, tag="pg", name=f"po{eq}_{tb}_{et}")
                            for mc in range(MT):
                                nc.tensor.matmul(
                                    po, lhsT=wos[j][:, mc, et * P : (et + 1) * P],
                                    rhs=lt[:, mc, :],
                                    start=(mc == 0), stop=(mc == MT - 1),
                                )
                            nc.vector.tensor_copy(ot[:, et, :], po)
                        nc.gpsimd.dma_start(
                            out=outT_r[:, eq * ETQ : (eq + 1) * ETQ, tsl], in_=ot
                        )
    nc.compile()
    return nc


_cached = {}


def _get_module(has_bias):
    if has_bias not in _cached:
        _cached[has_bias] = build_module(has_bias)
    return _cached[has_bias]


def _prep_inputs(x, Wv, Wg, bg, Wq, bq, Wo, has_bias):
    """Shard + lay out host-side. Returns per-core input dicts."""
    x = np.asarray(x, dtype=np.float32)
    Wvs = (np.asarray(Wv, dtype=np.float32) * SCALE).astype(np.float32)
    Wos = (np.asarray(Wo, dtype=np.float32) * SCALE).astype(np.float32)
    Wg = np.ascontiguousarray(Wg, dtype=np.float32)
    Wq = np.ascontiguousarray(Wq, dtype=np.float32)
    in_maps = []
    for c in range(N_CORES):
        b, h = c // 2, c % 2
        xTc = np.zeros((E, T), dtype=np.float32)
        start = h * OUT_T - HALO
        src = np.ascontiguousarray(x[b, max(start, 0) : h * OUT_T + OUT_T].T)
        xTc[:, T - src.shape[1] :] = src
        m = {"xT": xTc, "Wvs": Wvs, "Wg": Wg, "Wq": Wq, "Wos": Wos}
        if has_bias:
            m["bg"] = np.ascontiguousarray(bg, dtype=np.float32)
            m["bq"] = np.ascontiguousarray(bq, dtype=np.float32)
        in_maps.append(m)
    return in_maps


def run(x, Wv, Wg, bg, Wq, bq, Wo, trace=False):
    bg = np.asarray(bg, dtype=np.float32)
    bq = np.asarray(bq, dtype=np.float32)
    has_bias = bool(np.any(bg)) or bool(np.any(bq))
    nc = _get_module(has_bias)
    in_maps = _prep_inputs(x, Wv, Wg, bg, Wq, bq, Wo, has_bias)
    res = run_bass_kernel_spmd(
        nc, in_maps, core_ids=list(range(N_CORES)), trace=trace
    )
    out = np.empty((B, S, E), dtype=np.float32)
    for c in range(N_CORES):
        b, h = c // 2, c % 2
        out[b, h * OUT_T : (h + 1) * OUT_T] = res.results[c]["outT"].T
    return out, res


def kernel(**inputs):
    out, _ = run(**inputs)
    return out



# revision 2
# speedup vs baseline: 1.3835x; 1.3835x over previous
"""Trainium2 Bass kernel for nn_Decay (gated decay-memory block).

  gate  = sigmoid(x @ Wg + bg)
  store = (x @ Wv) * gate * scale          scale = sqrt(1 - decay)
  mem   = decay-scan(store)                y_t = store_t + decay * y_{t-1}
  que   = sigmoid(x @ Wq + bq)
  out   = (mem * que * scale) @ Wo

Sharding (8 cores): core c handles batch b = c//2, token half h = c%2
(2048 output tokens each).  The decay scan needs history: each core
computes a 256-token halo before its token range (zero-padded for h=0,
so all cores run the identical program).  decay^256 ~ 2e-6.  No
collectives.

Precision plan (tolerance rel 2e-2; predicted 1.39e-2 on real inputs):
 - V path and O path in bf16 (error contribution ~3e-3)
 - gate/que GEMMs: K rows 0..KF-1 in fp8 e4m3 with DoubleRow perf mode
   (1.44x PE rate), remainder rows in bf16.  Both fp8 operands are
   pre-scaled by 8 host-side (product 64) and the bf16 remainder weights
   by 64, so one PSUM accumulates 64*z; the sigmoid applies scale=1/64.
 - scan state/input fp32; que/l0/weights/x bf16; PSUM fp32; out fp32.

Layout: on-chip [feature (partitions), token (free)].  Free dim 512
(halo block 256) so f32-era LDWEIGHTS leak is amortized; bf16 gets FWL.

Schedule: 4 m-quarter phases x 5 token blocks; que-projection (pq) for
block i runs during block i+1 (and the last block's pq drains into the
next phase / the C transition), so wq loads and phase-boundary weight
loads always have a full block of PE work as cover.  Phase C (output
projection) keeps all four Wo e-quarters resident by reusing SBUF tag
space freed by the A-phase weights.
"""

import sys

sys.path.insert(0, "/opt/trn_rl_repo")

import ml_dtypes
import numpy as np

import concourse.bass as bass
import concourse.tile as tile
from concourse import bacc, mybir
from concourse.bass_utils import run_bass_kernel_spmd

# Problem constants (hardcoded per harness contract)
B, S, E, M = 4, 4096, 2048, 2048
DECAY = 0.95
SCALE = float(np.sqrt(1.0 - DECAY))

N_CORES = 8
HALO = 256
OUT_T = S // 2        # 2048 output tokens per core
T = OUT_T + HALO      # 2304 computed tokens per core
P = 128
KF = 1024             # fp8 K-prefix for gate/que projections
KR = E - KF
EC8 = KF // 256       # DoubleRow k-pair count (4)
ECR = KR // 128       # bf16 remainder k-chunks (8)
EC = E // P           # 16
MT = M // P           # 16
MQ = 4                # m-quarter phases
MT_Q = MT // MQ       # 4 m-tiles per quarter
MQW = MT_Q * P        # 512
BLK = [(0, 256), (256, 512), (768, 512), (1280, 512), (1792, 512)]
NB = len(BLK)
XS = 8.0              # fp8 per-operand scale (product 64)
WS = XS * XS
F32 = mybir.dt.float32
BF16 = mybir.dt.bfloat16
FP8 = mybir.dt.float8e4
DR = mybir.MatmulPerfMode.DoubleRow
SIG = mybir.ActivationFunctionType.Sigmoid


def build_module(has_bias):
    nc = bacc.Bacc()

    xT_d = nc.dram_tensor("xT16", [E, T], BF16, kind="ExternalInput")
    x8_d = nc.dram_tensor("xT8", [KF, T], FP8, kind="ExternalInput")
    wv_d = nc.dram_tensor("Wv16", [E, M], BF16, kind="ExternalInput")
    wg8_d = nc.dram_tensor("Wg8", [KF, M], FP8, kind="ExternalInput")
    wg16_d = nc.dram_tensor("Wg16", [KR, M], BF16, kind="ExternalInput")
    wq8_d = nc.dram_tensor("Wq8", [KF, M], FP8, kind="ExternalInput")
    wq16_d = nc.dram_tensor("Wq16", [KR, M], BF16, kind="ExternalInput")
    wo_d = nc.dram_tensor("Wo16", [M, E], BF16, kind="ExternalInput")
    if has_bias:
        bg_d = nc.dram_tensor("bg", [M], F32, kind="ExternalInput")
        bq_d = nc.dram_tensor("bq", [M], F32, kind="ExternalInput")
    outT_d = nc.dram_tensor("outT", [E, OUT_T], F32, kind="ExternalOutput")
    l0_d = nc.dram_tensor("l0buf", [M, OUT_T], BF16)  # internal spill

    xT_r = xT_d.rearrange("(c p) t -> p c t", p=P)
    x8_r = x8_d.rearrange("(c j p) t -> p c j t", p=P, j=2)
    l0_r = l0_d.rearrange("(c p) t -> p c t", p=P)
    outT_r = outT_d.rearrange("(c p) t -> p c t", p=P)

    with tile.TileContext(nc) as tc:
        with (
            tc.tile_pool(name="w", bufs=2) as wp,
            tc.tile_pool(name="a", bufs=2) as sp,
            tc.tile_pool(name="ps", bufs=2, space="PSUM") as ps,
        ):
            consts = sp.tile([P, 512 + 2 * MT], F32, tag="consts", bufs=1)
            nc.vector.memset(consts[:, 0:512], DECAY)
            if has_bias:
                nc.sync.dma_start(
                    out=consts[:, 512 : 512 + MT],
                    in_=bg_d.rearrange("(c p) -> p c", p=P),
                )
                nc.sync.dma_start(
                    out=consts[:, 512 + MT : 512 + 2 * MT],
                    in_=bq_d.rearrange("(c p) -> p c", p=P),
                )
            decay_t = consts[:, 0:512]

            def bias_ap(kind, mtg):
                if not has_bias:
                    return 0.0
                off = 512 + (0 if kind == "g" else MT) + mtg
                return consts[:, off : off + 1]

            W16 = [P, EC, MQW]  # 16KB/partition: wv / wo / (padded) wg16, wq16

            def load_wv(q):
                t = wp.tile(W16, BF16, tag="wv", name=f"wv{q}")
                nc.scalar.dma_start(
                    out=t,
                    in_=wv_d[:, q * MQW : (q + 1) * MQW].rearrange(
                        "(c p) m -> p c m", p=P
                    ),
                )
                return t

            def load_w8(d, q, tag, nm):
                t = wp.tile([P, EC8, 2, MQW], FP8, tag=tag, name=nm)
                nc.scalar.dma_start(
                    out=t,
                    in_=d[:, q * MQW : (q + 1) * MQW].rearrange(
                        "(c j p) m -> p c j m", p=P, j=2
                    ),
                )
                return t

            def load_w16(d, q, tag, nm):
                # allocated at full W16 size so wo quarters can reuse the tag
                t = wp.tile(W16, BF16, tag=tag, name=nm)
                nc.scalar.dma_start(
                    out=t[:, :ECR, :],
                    in_=d[:, q * MQW : (q + 1) * MQW].rearrange(
                        "(c p) m -> p c m", p=P
                    ),
                )
                return t

            def load_wo(eq, tag):
                t = wp.tile(W16, BF16, tag=tag, name=f"wo{eq}")
                nc.scalar.dma_start(
                    out=t,
                    in_=wo_d[:, eq * MQW : (eq + 1) * MQW].rearrange(
                        "(c p) e -> p c e", p=P
                    ),
                )
                return t

            def load_x(s):
                q, i = divmod(s, NB)
                t0, tsz = BLK[i]
                xt = sp.tile([P, EC, 512], BF16, tag="xt", bufs=3, name=f"xt{q}_{i}")
                nc.sync.dma_start(out=xt[:, :, :tsz], in_=xT_r[:, :, t0 : t0 + tsz])
                x8t = sp.tile(
                    [P, EC8, 2, 512], FP8, tag="x8", bufs=3, name=f"x8_{q}_{i}"
                )
                nc.sync.dma_start(
                    out=x8t[:, :, :, :tsz], in_=x8_r[:, :, :, t0 : t0 + tsz]
                )
                return xt, x8t

            def emit_pq(inf):
                # deferred que-projection + load for a previous block
                tszp = inf["tsz"]
                for mt in range(MT_Q):
                    msl = slice(mt * P, (mt + 1) * P)
                    mtg = inf["q"] * MT_Q + mt
                    pqp = ps.tile(
                        [P, 512], F32, tag="pq", bufs=2,
                        name=f"pq{inf['q']}_{inf['i']}_{mt}",
                    )[:, :tszp]
                    for c2 in range(EC8):
                        nc.tensor.matmul(
                            pqp, lhsT=inf["q8"][:, c2, :, msl],
                            rhs=inf["x8"][:, c2, :, :tszp],
                            start=(c2 == 0), stop=False, perf_mode=DR,
                        )
                    for ec in range(ECR):
                        nc.tensor.matmul(
                            pqp, lhsT=inf["q16"][:, ec, msl],
                            rhs=inf["xt"][:, 2 * EC8 + ec, :tszp],
                            start=False, stop=(ec == ECR - 1),
                        )
                    que = sp.tile(
                        [P, 512], BF16, tag="que", bufs=3,
                        name=f"que{inf['q']}_{inf['i']}_{mt}",
                    )[:, :tszp]
                    nc.scalar.activation(
                        que, pqp, SIG, bias=bias_ap("q", mtg), scale=1.0 / WS
                    )
                    l0 = sp.tile(
                        [P, 512], BF16, tag="l0", bufs=3,
                        name=f"l0_{inf['q']}_{inf['i']}_{mt}",
                    )[:, :tszp]
                    nc.vector.tensor_mul(l0, inf["mem"][:, mt, :tszp], que)
                    nc.gpsimd.dma_start(
                        out=l0_r[:, mtg : mtg + 1, inf["osl"]], in_=l0.unsqueeze(1)
                    )

            # ---- Phase A: 4 m-quarters x 5 token blocks ----
            steps = [(q, i) for q in range(MQ) for i in range(NB)]
            xts = {0: load_x(0)}
            cur = {
                "wv": load_wv(0),
                "g8": load_w8(wg8_d, 0, "wg8", "wg8_0"),
                "g16": load_w16(wg16_d, 0, "wg16", "wg16_0"),
                "q8": load_w8(wq8_d, 0, "wq8", "wq8_0"),
                "q16": load_w16(wq16_d, 0, "wq16", "wq16_0"),
            }
            nxt = {}
            wo_t = {}
            prev = None      # deferred-pq info from previous block
            mem_prev = None  # previous block's mem (scan chain)

            for s, (q, i) in enumerate(steps):
                t0, tsz = BLK[i]
                if i == 0 and q > 0:
                    cur = nxt
                    nxt = {}
                xt, x8t = xts.pop(s)
                if s + 1 < len(steps):
                    xts[s + 1] = load_x(s + 1)

                # phase-boundary weight prefetches (a quarter ahead / wo)
                if q < MQ - 1:
                    if i == 2:
                        nxt["wv"] = load_wv(q + 1)
                    elif i == 3:
                        nxt["g8"] = load_w8(wg8_d, q + 1, "wg8", f"wg8_{q+1}")
                        nxt["g16"] = load_w16(wg16_d, q + 1, "wg16", f"wg16_{q+1}")
                    elif i == 4:
                        nxt["q8"] = load_w8(wq8_d, q + 1, "wq8", f"wq8_{q+1}")
                        nxt["q16"] = load_w16(wq16_d, q + 1, "wq16", f"wq16_{q+1}")
                else:
                    if i == 1:
                        wo_t[2] = load_wo(2, "wq16")
                    elif i == 2:
                        wo_t[0] = load_wo(0, "wv")
                    elif i == 3:
                        wo_t[3] = load_wo(3, "wg16")

                # deferred pq for the previous block (keeps PE busy while
                # this block's x/weights stream in)
                if prev is not None:
                    emit_pq(prev)

                # pv + pg for this block, interleaved per m-tile
                mem_t = sp.tile(
                    [P, MT_Q, 512], F32, tag="mem", bufs=2, name=f"mem{q}_{i}"
                )
                for mt in range(MT_Q):
                    msl = slice(mt * P, (mt + 1) * P)
                    mtg = q * MT_Q + mt
                    pvp = ps.tile(
                        [P, 512], F32, tag="pv", bufs=3, name=f"pv{q}_{i}_{mt}"
                    )[:, :tsz]
                    for ec in range(EC):
                        nc.tensor.matmul(
                            pvp, lhsT=cur["wv"][:, ec, msl], rhs=xt[:, ec, :tsz],
                            start=(ec == 0), stop=(ec == EC - 1),
                        )
                    pgp = ps.tile(
                        [P, 512], F32, tag="pg", bufs=2, name=f"pg{q}_{i}_{mt}"
                    )[:, :tsz]
                    for c2 in range(EC8):
                        nc.tensor.matmul(
                            pgp, lhsT=cur["g8"][:, c2, :, msl],
                            rhs=x8t[:, c2, :, :tsz],
                            start=(c2 == 0), stop=False, perf_mode=DR,
                        )
                    for ec in range(ECR):
                        nc.tensor.matmul(
                            pgp, lhsT=cur["g16"][:, ec, msl],
                            rhs=xt[:, 2 * EC8 + ec, :tsz],
                            start=False, stop=(ec == ECR - 1),
                        )
                    gate = sp.tile(
                        [P, 512], BF16, tag="gate", bufs=2, name=f"gate{q}_{i}_{mt}"
                    )[:, :tsz]
                    nc.scalar.activation(
                        gate, pgp, SIG, bias=bias_ap("g", mtg), scale=1.0 / WS
                    )
                    store = sp.tile(
                        [P, 512], F32, tag="store", bufs=2, name=f"st{q}_{i}_{mt}"
                    )[:, :tsz]
                    nc.vector.tensor_mul(store, pvp, gate)
                    init = (
                        0.0
                        if i == 0
                        else mem_prev[:, mt, BLK[i - 1][1] - 1 : BLK[i - 1][1]]
                    )
                    nc.vector.tensor_tensor_scan(
                        mem_t[:, mt, :tsz], decay_t[:, :tsz], store,
                        initial=init,
                        op0=mybir.AluOpType.mult, op1=mybir.AluOpType.add,
                    )

                prev = (
                    None
                    if i == 0
                    else dict(
                        q=q, i=i, tsz=tsz, mem=mem_t, xt=xt, x8=x8t,
                        q8=cur["q8"], q16=cur["q16"],
                        osl=slice(t0 - HALO, t0 - HALO + tsz),
                    )
                )
                mem_prev = mem_t

            emit_pq(prev)  # drain: pq for (3, B4) covers the C transition
            wo_t[1] = load_wo(1, "wv")

            # ---- Phase C: output projection, all Wo quarters resident ----
            lt = sp.tile([P, MT, 512], BF16, tag="xt", bufs=3, name="lt0")
            nc.sync.dma_start(out=lt, in_=l0_r[:, :, 0:512])
            for tb in range(OUT_T // 512):
                tsl = slice(tb * 512, (tb + 1) * 512)
                lt_next = None
                if tb + 1 < OUT_T // 512:
                    lt_next = sp.tile(
                        [P, MT, 512], BF16, tag="xt", bufs=3, name=f"lt{tb+1}"
                    )
                    nc.sync.dma_start(
                        out=lt_next, in_=l0_r[:, :, (tb + 1) * 512 : (tb + 2) * 512]
                    )
                for eq in range(4):
                    ot = sp.tile(
                        [P, MT_Q, 512], F32, tag="mem", bufs=2, name=f"ot{eq}_{tb}"
                    )
                    for et in range(MT_Q):
                        pop = ps.tile(
                            [P, 512], F32, tag="pv", bufs=3, name=f"po{eq}_{tb}_{et}"
                        )
                        for mc in range(MT):
                            nc.tensor.matmul(
                                pop,
                                lhsT=wo_t[eq][:, mc, et * P : (et + 1) * P],
                                rhs=lt[:, mc, :],
                                start=(mc == 0), stop=(mc == MT - 1),
                            )
                        nc.vector.tensor_copy(ot[:, et, :], pop)
                    nc.gpsimd.dma_start(
                        out=outT_r[:, eq * MT_Q : (eq + 1) * MT_Q, tsl], in_=ot
                    )
                lt = lt_next
    nc.compile()
    return nc


_cached = {}


def _get_module(has_bias):
    if has_bias not in _cached:
        _cached[has_bias] = build_module(has_bias)
    return _cached[has_bias]


def _q8(a):
    return np.clip(a * np.float32(XS), -240, 240).astype(ml_dtypes.float8_e4m3)


def _prep_inputs(x, Wv, Wg, bg, Wq, bq, Wo, has_bias):
    """Shard + quantize host-side. Returns per-core input dicts."""
    bf = ml_dtypes.bfloat16
    x = np.asarray(x, dtype=np.float32)
    Wv16 = (np.asarray(Wv, np.float32) * np.float32(SCALE)).astype(bf)
    Wo16 = (np.asarray(Wo, np.float32) * np.float32(SCALE)).astype(bf)
    Wg = np.asarray(Wg, np.float32)
    Wq = np.asarray(Wq, np.float32)
    Wg8, Wq8 = _q8(Wg[:KF]), _q8(Wq[:KF])
    Wg16 = (Wg[KF:] * np.float32(WS)).astype(bf)
    Wq16 = (Wq[KF:] * np.float32(WS)).astype(bf)
    in_maps = []
    for c in range(N_CORES):
        b, h = c // 2, c % 2
        xTc = np.zeros((E, T), dtype=np.float32)
        start = h * OUT_T - HALO
        src = np.ascontiguousarray(x[b, max(start, 0) : h * OUT_T + OUT_T].T)
        xTc[:, T - src.shape[1] :] = src
        m = {
            "xT16": xTc.astype(bf), "xT8": _q8(xTc[:KF]),
            "Wv16": Wv16, "Wg8": Wg8, "Wg16": Wg16,
            "Wq8": Wq8, "Wq16": Wq16, "Wo16": Wo16,
        }
        if has_bias:
            m["bg"] = np.ascontiguousarray(bg, dtype=np.float32)
            m["bq"] = np.ascontiguousarray(bq, dtype=np.float32)
        in_maps.append(m)
    return in_maps


def run(x, Wv, Wg, bg, Wq, bq, Wo, trace=False):
    bg = np.asarray(bg, dtype=np.float32)
    bq = np.asarray(bq, dtype=np.float32)
    has_bias = bool(np.any(bg)) or bool(np.any(bq))
    nc = _get_module(has_bias)
    in_maps = _prep_inputs(x, Wv, Wg, bg, Wq, bq, Wo, has_bias)
    res = run_bass_kernel_spmd(
        nc, in_maps, core_ids=list(range(N_CORES)), trace=trace
    )
    out = np.empty((B, S, E), dtype=np.float32)
    for c in range(N_CORES):
        b, h = c // 2, c % 2
        out[b, h * OUT_T : (h + 1) * OUT_T] = res.results[c]["outT"].T
    return out, res


def kernel(**inputs):
    out, _ = run(**inputs)
    return out


# revision 7
# speedup vs baseline: 1.4828x; 1.0718x over previous
"""Trainium2 Bass kernel for nn_Decay (gated decay-memory block).

  gate  = sigmoid(x @ Wg + bg)
  store = (x @ Wv) * gate * scale          scale = sqrt(1 - decay)
  mem   = decay-scan(store)                y_t = store_t + decay * y_{t-1}
  que   = sigmoid(x @ Wq + bq)
  out   = (mem * que * scale) @ Wo

Sharding (8 cores): core c handles batch b = c//2, token half h = c%2
(2048 output tokens each).  The decay scan needs history: each core
computes a 256-token halo before its token range (zero-padded for h=0,
so all cores run the identical program).  decay^256 ~ 2e-6.  No
collectives.

Precision plan (tolerance rel 2e-2; predicted 1.39e-2 on real inputs):
 - V path and O path in bf16 (error contribution ~3e-3)
 - gate/que GEMMs: K rows 0..KF-1 in fp8 e4m3 with DoubleRow perf mode
   (1.44x PE rate), remainder rows in bf16.  Both fp8 operands are
   pre-scaled by 8 host-side (product 64) and the bf16 remainder weights
   by 64, so one PSUM accumulates 64*z; the sigmoid applies scale=1/64.
 - scan state/input fp32; que/l0/weights/x bf16; PSUM fp32; out fp32.

Layout: on-chip [feature (partitions), token (free)].  Free dim 512
(halo block 256) so f32-era LDWEIGHTS leak is amortized; bf16 gets FWL.

Schedule: 4 m-quarter phases x 5 token blocks; que-projection (pq) for
block i runs during block i+1 (and the last block's pq drains into the
next phase / the C transition), so wq loads and phase-boundary weight
loads always have a full block of PE work as cover.  Phase C (output
projection) keeps all four Wo e-quarters resident by reusing SBUF tag
space freed by the A-phase weights.
"""

import sys

sys.path.insert(0, "/opt/trn_rl_repo")

import ml_dtypes
import numpy as np

import concourse.bass as bass
import concourse.tile as tile
from concourse import bacc, mybir
from concourse.bass_utils import run_bass_kernel_spmd

# Problem constants (hardcoded per harness contract)
B, S, E, M = 4, 4096, 2048, 2048
DECAY = 0.95
SCALE = float(np.sqrt(1.0 - DECAY))

N_CORES = 8
HALO = 256
OUT_T = S // 2        # 2048 output tokens per core
T = OUT_T + HALO      # 2304 computed tokens per core
P = 128
KF = 1536             # fp8 K-prefix for gate/que projections
KR = E - KF
EC8 = KF // 256       # DoubleRow k-pair count (4)
ECR = KR // 128       # bf16 remainder k-chunks (8)
EC = E // P           # 16
MT = M // P           # 16
MQ = 4                # m-quarter phases
MT_Q = MT // MQ       # 4 m-tiles per quarter
MQW = MT_Q * P        # 512
BLK = [(0, 256), (256, 512), (768, 512), (1280, 512), (1792, 512)]
NB = len(BLK)
XS = 8.0              # fp8 per-operand scale (product 64)
WS = XS * XS
F32 = mybir.dt.float32
BF16 = mybir.dt.bfloat16
FP8 = mybir.dt.float8e4
DR = mybir.MatmulPerfMode.DoubleRow
SIG = mybir.ActivationFunctionType.Sigmoid


def build_module(has_bias):
    nc = bacc.Bacc()

    xT_d = nc.dram_tensor("xT16", [E, T], BF16, kind="ExternalInput")
    x8_d = nc.dram_tensor("xT8", [KF, T], FP8, kind="ExternalInput")
    wv_d = nc.dram_tensor("Wv16", [E, M], BF16, kind="ExternalInput")
    wg8_d = nc.dram_tensor("Wg8", [KF, M], FP8, kind="ExternalInput")
    wg16_d = nc.dram_tensor("Wg16", [KR, M], BF16, kind="ExternalInput")
    wq8_d = nc.dram_tensor("Wq8", [KF, M], FP8, kind="ExternalInput")
    wq16_d = nc.dram_tensor("Wq16", [KR, M], BF16, kind="ExternalInput")
    wo_d = nc.dram_tensor("Wo16", [M, E], BF16, kind="ExternalInput")
    if has_bias:
        bg_d = nc.dram_tensor("bg", [M], F32, kind="ExternalInput")
        bq_d = nc.dram_tensor("bq", [M], F32, kind="ExternalInput")
    outT_d = nc.dram_tensor("outT", [E, OUT_T], F32, kind="ExternalOutput")
    l0_d = nc.dram_tensor("l0buf", [M, OUT_T], BF16)  # internal spill

    xT_r = xT_d.rearrange("(c p) t -> p c t", p=P)
    x8_r = x8_d.rearrange("(c j p) t -> p c j t", p=P, j=2)
    l0_r = l0_d.rearrange("(c p) t -> p c t", p=P)
    outT_r = outT_d.rearrange("(c p) t -> p c t", p=P)

    with tile.TileContext(nc) as tc:
        with (
            tc.tile_pool(name="w", bufs=2) as wp,
            tc.tile_pool(name="a", bufs=2) as sp,
            tc.tile_pool(name="ps", bufs=2, space="PSUM") as ps,
        ):
            consts = sp.tile([P, 512 + 2 * MT], F32, tag="consts", bufs=1)
            nc.vector.memset(consts[:, 0:512], DECAY)
            if has_bias:
                nc.sync.dma_start(
                    out=consts[:, 512 : 512 + MT],
                    in_=bg_d.rearrange("(c p) -> p c", p=P),
                )
                nc.sync.dma_start(
                    out=consts[:, 512 + MT : 512 + 2 * MT],
                    in_=bq_d.rearrange("(c p) -> p c", p=P),
                )
            decay_t = consts[:, 0:512]

            def bias_ap(kind, mtg):
                if not has_bias:
                    return 0.0
                off = 512 + (0 if kind == "g" else MT) + mtg
                return consts[:, off : off + 1]

            W16 = [P, EC, MQW]  # 16KB/partition: wv / wo / (padded) wg16, wq16

            def load_wv(q):
                t = wp.tile(W16, BF16, tag="wv", name=f"wv{q}")
                nc.scalar.dma_start(
                    out=t,
                    in_=wv_d[:, q * MQW : (q + 1) * MQW].rearrange(
                        "(c p) m -> p c m", p=P
                    ),
                )
                return t

            def load_w8(d, q, tag, nm, eng=None):
                t = wp.tile([P, EC8, 2, MQW], FP8, tag=tag, name=nm)
                (eng or nc.scalar).dma_start(
                    out=t,
                    in_=d[:, q * MQW : (q + 1) * MQW].rearrange(
                        "(c j p) m -> p c j m", p=P, j=2
                    ),
                )
                return t

            def load_w16(d, q, tag, nm, eng=None):
                t = wp.tile([P, ECR, MQW], BF16, tag=tag, name=nm)
                (eng or nc.scalar).dma_start(
                    out=t,
                    in_=d[:, q * MQW : (q + 1) * MQW].rearrange(
                        "(c p) m -> p c m", p=P
                    ),
                )
                return t

            def load_wo(eq, tag):
                t = wp.tile(W16, BF16, tag=tag, name=f"wo{eq}")
                nc.scalar.dma_start(
                    out=t,
                    in_=wo_d[:, eq * MQW : (eq + 1) * MQW].rearrange(
                        "(c p) e -> p c e", p=P
                    ),
                )
                return t

            def load_x(s):
                q, i = divmod(s, NB)
                t0, tsz = BLK[i]
                xt = sp.tile([P, EC, 512], BF16, tag="xt", bufs=3, name=f"xt{q}_{i}")
                nc.sync.dma_start(out=xt[:, :, :tsz], in_=xT_r[:, :, t0 : t0 + tsz])
                x8t = sp.tile(
                    [P, EC8, 2, 512], FP8, tag="x8", bufs=3, name=f"x8_{q}_{i}"
                )
                nc.sync.dma_start(
                    out=x8t[:, :, :, :tsz], in_=x8_r[:, :, :, t0 : t0 + tsz]
                )
                return xt, x8t

            def emit_pq(inf):
                # deferred que-projection + load for a previous block
                tszp = inf["tsz"]
                for mt in range(MT_Q):
                    msl = slice(mt * P, (mt + 1) * P)
                    mtg = inf["q"] * MT_Q + mt
                    pqp = ps.tile(
                        [P, 512], F32, tag="pq", bufs=2,
                        name=f"pq{inf['q']}_{inf['i']}_{mt}",
                    )[:, :tszp]
                    for c2 in range(EC8):
                        nc.tensor.matmul(
                            pqp, lhsT=inf["q8"][:, c2, :, msl],
                            rhs=inf["x8"][:, c2, :, :tszp],
                            start=(c2 == 0), stop=False, perf_mode=DR,
                        )
                    for ec in range(ECR):
                        nc.tensor.matmul(
                            pqp, lhsT=inf["q16"][:, ec, msl],
                            rhs=inf["xt"][:, 2 * EC8 + ec, :tszp],
                            start=False, stop=(ec == ECR - 1),
                        )
                    que = sp.tile(
                        [P, 512], BF16, tag="que", bufs=3,
                        name=f"que{inf['q']}_{inf['i']}_{mt}",
                    )[:, :tszp]
                    nc.scalar.activation(
                        que, pqp, SIG, bias=bias_ap("q", mtg), scale=1.0 / WS
                    )
                    l0 = sp.tile(
                        [P, 512], BF16, tag="l0", bufs=3,
                        name=f"l0_{inf['q']}_{inf['i']}_{mt}",
                    )[:, :tszp]
                    nc.vector.tensor_mul(l0, inf["mem"][:, mt, :tszp], que)
                    nc.gpsimd.dma_start(
                        out=l0_r[:, mtg : mtg + 1, inf["osl"]], in_=l0.unsqueeze(1)
                    )

            # ---- Phase A: 4 m-quarters x 5 token blocks ----
            steps = [(q, i) for q in range(MQ) for i in range(NB)]
            xts = {0: load_x(0)}
            # startup: spread the q0 weight loads over distinct DMA queues so
            # the first blocks' pv/pg aren't serialized behind one queue
            cur = {
                "wv": load_wv(0),
                "g8": load_w8(wg8_d, 0, "wg8", "wg8_0", nc.gpsimd),
                "g16": load_w16(wg16_d, 0, "wg16", "wg16_0", nc.gpsimd),
                "q8": load_w8(wq8_d, 0, "wq8", "wq8_0", nc.gpsimd),
                "q16": load_w16(wq16_d, 0, "wq16", "wq16_0", nc.gpsimd),
            }
            nxt = {}
            wo_t = {}
            prev = None      # deferred-pq info from previous block
            mem_prev = None  # previous block's mem (scan chain)

            for s, (q, i) in enumerate(steps):
                t0, tsz = BLK[i]
                if i == 0 and q > 0:
                    cur = nxt
                    nxt = {}
                xt, x8t = xts.pop(s)
                if s + 1 < len(steps):
                    xts[s + 1] = load_x(s + 1)

                # phase-boundary weight prefetches (a quarter ahead / wo)
                if q < MQ - 1:
                    if i == 2:
                        nxt["wv"] = load_wv(q + 1)
                    elif i == 3:
                        nxt["g8"] = load_w8(wg8_d, q + 1, "wg8", f"wg8_{q+1}")
                        nxt["g16"] = load_w16(wg16_d, q + 1, "wg16", f"wg16_{q+1}")
                    elif i == 4:
                        nxt["q8"] = load_w8(wq8_d, q + 1, "wq8", f"wq8_{q+1}")
                        nxt["q16"] = load_w16(wq16_d, q + 1, "wq16", f"wq16_{q+1}")
                else:
                    if i == 1:
                        wo_t[2] = load_wo(2, "wo")
                    elif i == 2:
                        wo_t[0] = load_wo(0, "wv")
                    elif i == 3:
                        wo_t[3] = load_wo(3, "wo")

                # deferred pq for the previous block (keeps PE busy while
                # this block's x/weights stream in)
                if prev is not None:
                    emit_pq(prev)

                # pv + pg for this block, interleaved per m-tile
                mem_t = sp.tile(
                    [P, MT_Q, 512], F32, tag="mem", bufs=2, name=f"mem{q}_{i}"
                )
                for mt in range(MT_Q):
                    msl = slice(mt * P, (mt + 1) * P)
                    mtg = q * MT_Q + mt
                    pvp = ps.tile(
                        [P, 512], F32, tag="pv", bufs=3, name=f"pv{q}_{i}_{mt}"
                    )[:, :tsz]
                    for ec in range(EC):
                        nc.tensor.matmul(
                            pvp, lhsT=cur["wv"][:, ec, msl], rhs=xt[:, ec, :tsz],
                            start=(ec == 0), stop=(ec == EC - 1),
                        )
                    pgp = ps.tile(
                        [P, 512], F32, tag="pg", bufs=2, name=f"pg{q}_{i}_{mt}"
                    )[:, :tsz]
                    for c2 in range(EC8):
                        nc.tensor.matmul(
                            pgp, lhsT=cur["g8"][:, c2, :, msl],
                            rhs=x8t[:, c2, :, :tsz],
                            start=(c2 == 0), stop=False, perf_mode=DR,
                        )
                    for ec in range(ECR):
                        nc.tensor.matmul(
                            pgp, lhsT=cur["g16"][:, ec, msl],
                            rhs=xt[:, 2 * EC8 + ec, :tsz],
                            start=False, stop=(ec == ECR - 1),
                        )
                    gate = sp.tile(
                        [P, 512], BF16, tag="gate", bufs=2, name=f"gate{q}_{i}_{mt}"
                    )[:, :tsz]
                    nc.scalar.activation(
                        gate, pgp, SIG, bias=bias_ap("g", mtg), scale=1.0 / WS
                    )
                    store = sp.tile(
                        [P, 512], F32, tag="store", bufs=2, name=f"st{q}_{i}_{mt}"
                    )[:, :tsz]
                    nc.vector.tensor_mul(store, pvp, gate)
                    init = (
                        0.0
                        if i == 0
                        else mem_prev[:, mt, BLK[i - 1][1] - 1 : BLK[i - 1][1]]
                    )
                    nc.vector.tensor_tensor_scan(
                        mem_t[:, mt, :tsz], decay_t[:, :tsz], store,
                        initial=init,
                        op0=mybir.AluOpType.mult, op1=mybir.AluOpType.add,
                    )

                prev = (
                    None
                    if i == 0
                    else dict(
                        q=q, i=i, tsz=tsz, mem=mem_t, xt=xt, x8=x8t,
                        q8=cur["q8"], q16=cur["q16"],
                        osl=slice(t0 - HALO, t0 - HALO + tsz),
                    )
                )
                mem_prev = mem_t

            emit_pq(prev)  # drain: pq for (3, B4) covers the C transition
            wo_t[1] = load_wo(1, "wv")

            # ---- Phase C: output projection, all Wo quarters resident ----
            lt = sp.tile([P, MT, 512], BF16, tag="xt", bufs=3, name="lt0")
            nc.sync.dma_start(out=lt, in_=l0_r[:, :, 0:512])
            for tb in range(OUT_T // 512):
                tsl = slice(tb * 512, (tb + 1) * 512)
                lt_next = None
                if tb + 1 < OUT_T // 512:
                    lt_next = sp.tile(
                        [P, MT, 512], BF16, tag="xt", bufs=3, name=f"lt{tb+1}"
                    )
                    nc.sync.dma_start(
                        out=lt_next, in_=l0_r[:, :, (tb + 1) * 512 : (tb + 2) * 512]
                    )
                for eq in range(4):
                    ot = sp.tile(
                        [P, MT_Q, 512], F32, tag="mem", bufs=2, name=f"ot{eq}_{tb}"
                    )
                    for et in range(MT_Q):
                        pop = ps.tile(
                            [P, 512], F32, tag="pv", bufs=3, name=f"po{eq}_{tb}_{et}"
                        )
                        for mc in range(MT):
                            nc.tensor.matmul(
                                pop,
                                lhsT=wo_t[eq][:, mc, et * P : (et + 1) * P],
                                rhs=lt[:, mc, :],
                                start=(mc == 0), stop=(mc == MT - 1),
                            )
                        nc.vector.tensor_copy(ot[:, et, :], pop)
                    nc.gpsimd.dma_start(
                        out=outT_r[:, eq * MT_Q : (eq + 1) * MT_Q, tsl], in_=ot
                    )
                lt = lt_next
    nc.compile()
    return nc


_cached = {}


def _get_module(has_bias):
    if has_bias not in _cached:
        _cached[has_bias] = build_module(has_bias)
    return _cached[has_bias]


def _q8(a):
    return np.clip(a * np.float32(XS), -240, 240).astype(ml_dtypes.float8_e4m3)


def _prep_inputs(x, Wv, Wg, bg, Wq, bq, Wo, has_bias):
    """Shard + quantize host-side. Returns per-core input dicts."""
    bf = ml_dtypes.bfloat16
    x = np.asarray(x, dtype=np.float32)
    Wv16 = (np.asarray(Wv, np.float32) * np.float32(SCALE)).astype(bf)
    Wo16 = (np.asarray(Wo, np.float32) * np.float32(SCALE)).astype(bf)
    Wg = np.asarray(Wg, np.float32)
    Wq = np.asarray(Wq, np.float32)
    Wg8, Wq8 = _q8(Wg[:KF]), _q8(Wq[:KF])
    Wg16 = (Wg[KF:] * np.float32(WS)).astype(bf)
    Wq16 = (Wq[KF:] * np.float32(WS)).astype(bf)
    in_maps = []
    for c in range(N_CORES):
        b, h = c // 2, c % 2
        xTc = np.zeros((E, T), dtype=np.float32)
        start = h * OUT_T - HALO
        src = np.ascontiguousarray(x[b, max(start, 0) : h * OUT_T + OUT_T].T)
        xTc[:, T - src.shape[1] :] = src
        m = {
            "xT16": xTc.astype(bf), "xT8": _q8(xTc[:KF]),
            "Wv16": Wv16, "Wg8": Wg8, "Wg16": Wg16,
            "Wq8": Wq8, "Wq16": Wq16, "Wo16": Wo16,
        }
        if has_bias:
            m["bg"] = np.ascontiguousarray(bg, dtype=np.float32)
            m["bq"] = np.ascontiguousarray(bq, dtype=np.float32)
        in_maps.append(m)
    return in_maps


def run(x, Wv, Wg, bg, Wq, bq, Wo, trace=False):
    bg = np.asarray(bg, dtype=np.float32)
    bq = np.asarray(bq, dtype=np.float32)
    has_bias = bool(np.any(bg)) or bool(np.any(bq))
    nc = _get_module(has_bias)
    in_maps = _prep_inputs(x, Wv, Wg, bg, Wq, bq, Wo, has_bias)
    res = run_bass_kernel_spmd(
        nc, in_maps, core_ids=list(range(N_CORES)), trace=trace
    )
    out = np.empty((B, S, E), dtype=np.float32)
    for c in range(N_CORES):
        b, h = c // 2, c % 2
        out[b, h * OUT_T : (h + 1) * OUT_T] = res.results[c]["outT"].T
    return out, res


def kernel(**inputs):
    out, _ = run(**inputs)
    return out
